# revision 1
# baseline (speedup 1.0000x reference)
"""DCNv2 (modulated deformable 3x3 conv) + GroupNorm fused Trainium2 kernel.

Strategy (data-parallel over batch, 1 sample per NeuronCore):
  - x[b] is passed transposed+padded as XT [4098, 256] bf16 in DRAM.
  - On-device: compute bilinear sample indices + per-tap weights (incl. mask
    modulation and zero-padding validity) from offset/mask with DVE ops in a
    packed [72, 512] layout (partition q = k*8 + T, T = 512-pixel tile).
  - Gather: one indirect DMA per (k, T): 1024 row-pair descriptors (top/bottom
    bilinear rows, each 512 contiguous bf16 = 2 pixels x 256 channels).
  - Combine 4 taps with per-pixel weights via DVE scalar_tensor_tensor in
    [pixel-partition, channel-free] layout (weights are per-partition scalars,
    PE-transposed into place).
  - PE-transpose val back to [channel, pixel], implicit-GEMM over (c,k) with
    bf16 weights accumulating in PSUM, GroupNorm stats on the fly, final
    normalize + affine on the second pass over the (small) output.
"""
import sys, os

sys.path.insert(0, "/opt/trn_rl_repo")

KSTAGE = int(os.environ.get("KSTAGE", "7"))
KSTATS = int(os.environ.get("KSTATS", "1"))

import numpy as np
import ml_dtypes

import concourse.bass as bass
import concourse.tile as tile
from concourse import bacc, mybir
from concourse.bass_utils import run_bass_kernel_spmd

f32 = mybir.dt.float32
bf16 = mybir.dt.bfloat16
i32 = mybir.dt.int32
i16 = mybir.dt.int16
u8 = mybir.dt.uint8
alu = mybir.AluOpType
act = mybir.ActivationFunctionType

B, C, O, H, W = 8, 256, 256, 64, 64
HW = H * W
K = 9
GROUPS = 16
EPS = 1e-5
NT = 8          # pixel tiles per image
TS = 512        # pixels per tile
Q = K * NT      # 72 packed rows
NPERG = (O // GROUPS) * HW  # elements per group = 16*4096


def _emit(nc, tc):
    xt = nc.declare_dram_parameter("xt", [HW + 2, C], bf16, isOutput=False)
    off = nc.declare_dram_parameter("off", [18, HW], f32, isOutput=False)
    msk = nc.declare_dram_parameter("msk", [K, HW], f32, isOutput=False)
    wT = nc.declare_dram_parameter("wT", [18, 128, O], bf16, isOutput=False)
    gbp = nc.declare_dram_parameter("gb", [O, 2], f32, isOutput=False)
    cby = nc.declare_dram_parameter("cby", [Q, TS], f32, isOutput=False)
    cbx = nc.declare_dram_parameter("cbx", [Q, TS], f32, isOutput=False)
    g16 = nc.declare_dram_parameter("g16", [128, 8], f32, isOutput=False)
    g16t = nc.declare_dram_parameter("g16t", [8, 128], f32, isOutput=False)
    idn = nc.declare_dram_parameter("idn", [128, 128], bf16, isOutput=False)
    idnf = nc.declare_dram_parameter("idnf", [128, 128], f32, isOutput=False)
    idni = nc.declare_dram_parameter("idni", [128, 128], u8, isOutput=False)
    outp = nc.declare_dram_parameter("out", [O, HW], f32, isOutput=True)

    dv = nc.vector
    sc = nc.scalar
    pe = nc.tensor
    gs = nc.gpsimd

    with (
        tc.tile_pool(name="const", bufs=1) as constp,
        tc.tile_pool(name="math", bufs=1) as mathp,
        tc.tile_pool(name="gat", bufs=3) as gatp,
        tc.tile_pool(name="val", bufs=2) as valp,
        tc.tile_pool(name="vts", bufs=2) as vtsp,
        tc.tile_pool(name="big", bufs=1) as bigp,
        tc.tile_pool(name="ps", bufs=2, space="PSUM") as psp,
        tc.tile_pool(name="psw", bufs=1, space="PSUM") as pswp,
        tc.tile_pool(name="pst", bufs=2, space="PSUM") as pstp,
    ):
        # ---- constant loads ----
        wTs = constp.tile([128, 18, O], bf16)
        gs.dma_start(wTs[:], wT.ap().transpose([1, 0, 2]))
        idnb = constp.tile([128, 128], bf16)
        gs.dma_start(idnb[:], idn.ap())
        idnft = constp.tile([128, 128], f32)
        gs.dma_start(idnft[:], idnf.ap())
        idnis = constp.tile([128, 128], u8)
        gs.dma_start(idnis[:], idni.ap())
        g16s = constp.tile([128, 8], f32)
        gs.dma_start(g16s[:], g16.ap())
        g16ts = constp.tile([8, 128], f32)
        gs.dma_start(g16ts[:], g16t.ap())
        gbs = constp.tile([128, 2, 2], f32)
        gs.dma_start(gbs[:], gbp.ap().rearrange("(m p) two -> p m two", m=2))

        # ---- packed [72, 512] loads of dy/dx/mask and index ramps ----
        def packed_load(name, src_ap):
            t = mathp.tile([Q, TS], f32, tag=name, name=name)
            gs.dma_start(t[:], src_ap)
            return t

        offv = off.ap().rearrange("(k two) (t s) -> two k t s", two=2, t=NT)
        dys = packed_load("dys", offv[0])
        dxs = packed_load("dxs", offv[1])
        msks = packed_load("msks", msk.ap().rearrange("k (t s) -> k t s", t=NT))
        cbys = packed_load("cbys", cby.ap())
        cbxs = packed_load("cbxs", cbx.ap())

        # Scratch-tag aliasing: transient [72,512] temporaries share slots
        # (Tile inserts WAR deps on reuse; all are sequential DVE ops anyway).
        TAGMAP = {
            "ys": "tA", "yi": "ti", "yf": "tB", "yo": "tC",
            "xs": "tA", "xi": "ti", "xf": "tB", "xo": "tC",
            "yb": "tA", "xlc": "tB", "xrc": "tC",
            "vt": "u1", "vb": "u2", "vl": "u3", "vr": "u4",
            "wyt": "tD", "wxl": "tE",
            "wa": "u5", "wb": "u6",
            "cl": "u1", "cm": "u3", "dr": "u2", "dm": "u4",
            "flat": "tA", "flat2": "tB",
        }

        def mtile(tag, dt=f32):
            tag = TAGMAP.get(tag, tag)
            return mathp.tile([Q, TS], dt, tag=tag, name=tag)

        # ---- floor + frac (robust to cast rounding mode) ----
        def floor_frac(base, d, pre):
            s = mtile(pre + "s")
            dv.tensor_tensor(out=s[:], in0=base[:], in1=d[:], op=alu.add)
            ii = mtile(pre + "i", i32)
            dv.tensor_copy(ii[:], s[:])
            ff = mtile(pre + "f")
            dv.tensor_copy(ff[:], ii[:])
            ov = mtile(pre + "o")
            dv.tensor_tensor(out=ov[:], in0=ff[:], in1=s[:], op=alu.is_gt)
            f0 = mtile(pre + "0")
            dv.tensor_tensor(out=f0[:], in0=ff[:], in1=ov[:], op=alu.subtract)
            fr = mtile(pre + "r")
            dv.tensor_tensor(out=fr[:], in0=s[:], in1=f0[:], op=alu.subtract)
            return f0, fr  # integer part (shifted by +16), fraction in [0,1)

        y0, wy = floor_frac(cbys, dys, "y")
        x0, wx = floor_frac(cbxs, dxs, "x")

        def clamp(src, lo, hi, tag):
            t = mtile(tag)
            dv.tensor_scalar(t[:], src[:], float(lo), float(hi), alu.max, alu.min)
            return t

        y0c = clamp(y0, 16, 79, "y0c")
        yb = mtile("yb")
        dv.tensor_scalar(yb[:], y0[:], 1.0, None, alu.add)
        ybc = clamp(yb, 16, 79, "ybc")
        x0c = clamp(x0, 15, 79, "x0c")   # gather clamp (real -1 allowed: R tap)
        xlc = clamp(x0, 16, 79, "xlc")   # left-tap validity clamp
        xrc = clamp(x0, 15, 78, "xrc")   # right-tap validity clamp

        def is_eq(a, b, tag):
            t = mtile(tag)
            dv.tensor_tensor(out=t[:], in0=a[:], in1=b[:], op=alu.is_equal)
            return t

        vt = is_eq(y0, y0c, "vt")
        vb = is_eq(yb, ybc, "vb")
        vl = is_eq(x0, xlc, "vl")
        vr = is_eq(x0, xrc, "vr")

        wyt = mtile("wyt")
        dv.tensor_scalar(wyt[:], wy[:], -1.0, 1.0, alu.mult, alu.add)
        wxl = mtile("wxl")
        dv.tensor_scalar(wxl[:], wx[:], -1.0, 1.0, alu.mult, alu.add)

        def tmul(a, b, tag):
            t = mtile(tag)
            dv.tensor_tensor(out=t[:], in0=a[:], in1=b[:], op=alu.mult)
            return t

        wa = tmul(wyt, vt, "wa")      # top row weight * validity
        wb = tmul(wy, vb, "wb")       # bottom
        cl = tmul(wxl, vl, "cl")
        cm = tmul(cl, msks, "cm")     # left col weight * validity * mask
        dr = tmul(wx, vr, "dr")
        dm = tmul(dr, msks, "dm")
        w4 = [
            tmul(wa, cm, "wtl"),
            tmul(wa, dm, "wtr"),
            tmul(wb, cm, "wbl"),
            tmul(wb, dm, "wbr"),
        ]

        # ---- flat pair-row indices (with +1 lead-pad row) ----
        # dma_gather wants idx i (= tb*512 + t) at wrapped position
        # [pp = i%16, col = i//16 = tb*32 + t//16], int16, replicated across
        # all eight 16-partition groups.  Write the permuted row on DVE,
        # bounce through DRAM, reload replicated as [128, Q, 64].
        idx16 = bigp.tile([Q, 2 * TS], i16)
        idx16v = idx16[:].rearrange("q (pp s) -> q pp s", pp=16)
        for row, ysrc in ((0, y0c), (1, ybc)):
            ftmp = mtile("flat")
            dv.scalar_tensor_tensor(
                out=ftmp[:], in0=ysrc[:], scalar=64.0, in1=x0c[:],
                op0=alu.mult, op1=alu.add,
            )
            f2 = mtile("flat2")
            dv.tensor_scalar(f2[:], ftmp[:], -1039.0, None, alu.add)
            dv.tensor_copy(idx16v[:, :, row * 32:(row + 1) * 32],
                           f2[:].rearrange("q (a pp) -> q pp a", pp=16))
        idxd = nc.dram_tensor("idxd", [Q, 2 * TS], i16)
        gs.dma_start(idxd.ap(), idx16[:])
        idxw = bigp.tile([128, Q, 64], i16)
        idxdv = idxd.ap().rearrange("q (pp s) -> pp q s", pp=16)
        for grp in range(8):
            gs.dma_start(idxw[grp * 16:(grp + 1) * 16], idxdv)

        # ---- transpose tap weights to [pixel-in-128, (tap, j, q)] ----
        wts = bigp.tile([128, 4, 4, Q], f32)
        for t in range(4):
            for j in range(4):
                pw = pswp.tile([128, Q], f32, tag="pw")
                pe.transpose(pw[:], w4[t][:, j * 128:(j + 1) * 128], idnft[:Q, :Q])
                sc.activation(wts[:, t, j, :], pw[:], act.Copy)
        wtsb = bigp.tile([128, 4, 4, Q], bf16)
        dv.tensor_copy(wtsb[:], wts[:])
        # Persistent ping-pong diag tiles; off-diagonal zeroed once, the
        # per-(k,T) copy_predicated only rewrites diagonal entries.
        dt0 = bigp.tile([128, 128, 16], bf16)
        dt1 = bigp.tile([128, 128, 16], bf16)
        gs.memset(dt0[:], 0.0)
        gs.memset(dt1[:], 0.0)
        dts = (dt0, dt1)

        # ---- stats accumulators ----
        stats = bigp.tile([128, 2, 2, NT], f32)
        out_sb = bigp.tile([128, 2, HW], f32)
        sqscr = mathp.tile([128, TS], f32, tag="sqscr")
        dv.memset(sqscr[:], 0.0)

        if KSTAGE < 6:
            dv.memset(out_sb[:], 0.0)
        # ---- main loop ----
        for T in range(NT):
            vts = vtsp.tile([128, K, 2, TS], bf16, tag="vts")
            if KSTAGE >= 5:
                pass
            for k in range(K if KSTAGE >= 3 else 0):
                q = k * NT + T
                g = gatp.tile([128, 8, TS], bf16, tag="g")
                gs.dma_gather(
                    out_ap=g[:],
                    in_ap=bass.AP(xt.ap().tensor, 0, [[256, 4097], [1, 512]]),
                    idxs_ap=idxw[:, q, :],
                    num_idxs=1024, num_idxs_reg=1024,
                    elem_size=512, elem_step=256,
                )
                if KSTAGE < 4:
                    continue
                dt = dts[(T * K + k) % 2]
                dv.copy_predicated(
                    dt[:].rearrange("p c t -> p t c"),
                    idnis[:].unsqueeze(1).broadcast_to([128, 16, 128]),
                    wtsb[:, :, :, q].rearrange("p a b -> p (a b)")
                        .unsqueeze(2).broadcast_to([128, 16, 128]),
                )
                if KSTAGE < 5:
                    continue
                for ch in range(2):
                    psA = pstp.tile([128, 4, 128], f32, tag="pst")
                    for j in range(4):
                        for t in range(4):
                            lhsT = g[:, (t // 2) * 4 + j,
                                     (t % 2) * 256 + ch * 128:
                                     (t % 2) * 256 + (ch + 1) * 128]
                            pe.matmul(psA[:, j, :], lhsT, dt[:, :, t * 4 + j],
                                      start=(t == 0), stop=(t == 3))
                    sc.activation(vts[:, k, ch, :],
                                  psA[:].rearrange("p a b -> p (a b)"), act.Copy)
            for m in range(2 if KSTAGE >= 6 else 0):
                pso = psp.tile([128, TS], f32, tag="pso")
                for i in range(18):
                    k, ch = i // 2, i % 2
                    pe.matmul(
                        pso[:],
                        wTs[:, i, m * 128:(m + 1) * 128],
                        vts[:, k, ch, :],
                        start=(i == 0),
                        stop=(i == 17),
                    )
                osl = out_sb[:, m, T * TS:(T + 1) * TS]
                sc.activation(osl, pso[:], act.Copy)
                if KSTATS:
                    dv.tensor_reduce(stats[:, m, 0, T:T + 1], osl,
                                     mybir.AxisListType.X, alu.add)
                    dv.tensor_tensor(out=sqscr[:], in0=osl, in1=osl, op=alu.mult)
                    dv.tensor_reduce(stats[:, m, 1, T:T + 1], sqscr[:],
                                     mybir.AxisListType.X, alu.add)

        if KSTAGE < 7:
            for m in range(2):
                for T in range(NT):
                    gs.dma_start(outp.ap()[m * 128:(m + 1) * 128,
                                           T * TS:(T + 1) * TS],
                                 out_sb[:, m, T * TS:(T + 1) * TS])
        # ---- finalize GroupNorm ----
        for m in range(2 if KSTAGE >= 7 else 0):
            tot = mathp.tile([128, 2], f32, tag="tot")
            dv.tensor_reduce(tot[:], stats[:, m, :, :], mybir.AxisListType.X, alu.add)
            psg = pswp.tile([8, 2], f32, tag="psg")
            pe.matmul(psg[:], g16s[:], tot[:], start=True, stop=True)
            sg = mathp.tile([8, 2], f32, tag="sg")
            dv.tensor_copy(sg[:], psg[:])
            mug = mathp.tile([8, 2], f32, tag="mug")  # [:,0]=mu, [:,1]=rs
            dv.tensor_scalar(mug[:, 0:1], sg[:, 0:1], 1.0 / NPERG, None, alu.mult)
            msq = mathp.tile([8, 1], f32, tag="msq")
            dv.tensor_scalar(msq[:], sg[:, 1:2], 1.0 / NPERG, None, alu.mult)
            var = mathp.tile([8, 1], f32, tag="var")
            dv.tensor_tensor(out=var[:], in0=mug[:, 0:1], in1=mug[:, 0:1],
                             op=alu.mult)
            dv.tensor_tensor(out=var[:], in0=msq[:], in1=var[:], op=alu.subtract)
            dv.tensor_scalar(var[:], var[:], EPS, None, alu.add)
            sd = mathp.tile([8, 1], f32, tag="sd", name="sd")
            sc.activation(sd[:], var[:], act.Sqrt)
            dv.reciprocal(mug[:, 1:2], sd[:])
            pse = pswp.tile([128, 2], f32, tag="pse")
            pe.matmul(pse[:], g16ts[:], mug[:], start=True, stop=True)
            ce = mathp.tile([128, 2], f32, tag="ce")
            dv.tensor_copy(ce[:], pse[:])
            scv = mathp.tile([128, 1], f32, tag="scv")
            dv.tensor_tensor(out=scv[:], in0=ce[:, 1:2], in1=gbs[:, m, 0:1],
                             op=alu.mult)
            shv = mathp.tile([128, 1], f32, tag="shv")
            dv.tensor_tensor(out=shv[:], in0=ce[:, 0:1], in1=scv[:], op=alu.mult)
            dv.tensor_tensor(out=shv[:], in0=gbs[:, m, 1:2], in1=shv[:],
                             op=alu.subtract)
            for T in range(NT):
                fin = mathp.tile([128, TS], f32, tag="fin")
                dv.scalar_tensor_tensor(
                    out=fin[:], in0=out_sb[:, m, T * TS:(T + 1) * TS],
                    scalar=scv[:], in1=shv[:].broadcast_to([128, TS]),
                    op0=alu.mult, op1=alu.add,
                )
                gs.dma_start(outp.ap()[m * 128:(m + 1) * 128,
                                       T * TS:(T + 1) * TS], fin[:])


def build_nc():
    nc = bacc.Bacc("TRN2", target_bir_lowering=False, debug=False, num_devices=B)
    with tile.TileContext(nc) as tc:
        _emit(nc, tc)
    nc.compile()
    return nc


def host_inputs(x, offset, mask, weight, gamma, beta):
    """Per-core input maps (layout transforms only)."""
    wt = (
        weight.reshape(O, C, K).transpose(2, 1, 0)
        .reshape(K, 2, 128, O).reshape(18, 128, O)
        .astype(ml_dtypes.bfloat16)
    )
    gb = np.stack([gamma, beta], axis=-1).astype(np.float32)
    n = np.arange(HW)
    h = n // W
    w = n % W
    kk = np.arange(K)
    ky = kk // 3 - 1
    kx = kk % 3 - 1
    cby = (h[None, :] + ky[:, None] + 16.0).astype(np.float32).reshape(Q, TS)
    cbx = (w[None, :] + kx[:, None] + 16.0).astype(np.float32).reshape(Q, TS)
    ii = np.arange(128)
    g16 = (ii[:, None] // 16 == np.arange(8)[None, :]).astype(np.float32)
    g16t = np.ascontiguousarray(g16.T)
    idn = np.eye(128, dtype=ml_dtypes.bfloat16)
    idnf = np.eye(128, dtype=np.float32)
    maps = []
    for b in range(B):
        xtb = np.zeros((HW + 2, C), dtype=ml_dtypes.bfloat16)
        xtb[1:HW + 1] = x[b].reshape(C, HW).T.astype(ml_dtypes.bfloat16)
        maps.append({
            "xt": xtb,
            "off": np.ascontiguousarray(offset[b].reshape(18, HW), np.float32),
            "msk": np.ascontiguousarray(mask[b].reshape(K, HW), np.float32),
            "wT": wt, "gb": gb, "cby": cby, "cbx": cbx,
            "g16": g16, "g16t": g16t, "idn": idn, "idnf": idnf,
            "idni": np.eye(128, dtype=np.uint8),
        })
    return maps


def kernel(x, offset, mask, weight, gamma, beta):
    x = np.asarray(x)
    offset = np.asarray(offset)
    mask = np.asarray(mask)
    weight = np.asarray(weight)
    gamma = np.asarray(gamma)
    beta = np.asarray(beta)
    nc = build_nc()
    maps = host_inputs(x, offset, mask, weight, gamma, beta)
    res = run_bass_kernel_spmd(nc, maps, list(range(B)))
    out = np.stack([res.results[b]["out"] for b in range(B)], axis=0)
    return out.reshape(B, O, H, W).astype(np.float32)


if __name__ == "__main__":
    pass



# revision 2
# speedup vs baseline: 24.9178x; 24.9178x over previous
"""DCNv2 (modulated deformable 3x3 conv) + GroupNorm fused Trainium2 kernel.

Strategy (data-parallel over batch, 1 sample per NeuronCore):
  - x[b] is passed transposed+padded as XT [4098, 256] bf16 in DRAM.
  - On-device: compute bilinear sample indices + per-tap weights (incl. mask
    modulation and zero-padding validity) from offset/mask with DVE ops in a
    packed [72, 512] layout (partition q = k*8 + T, T = 512-pixel tile).
  - Gather: one indirect DMA per (k, T): 1024 row-pair descriptors (top/bottom
    bilinear rows, each 512 contiguous bf16 = 2 pixels x 256 channels).
  - Combine 4 taps with per-pixel weights via DVE scalar_tensor_tensor in
    [pixel-partition, channel-free] layout (weights are per-partition scalars,
    PE-transposed into place).
  - PE-transpose val back to [channel, pixel], implicit-GEMM over (c,k) with
    bf16 weights accumulating in PSUM, GroupNorm stats on the fly, final
    normalize + affine on the second pass over the (small) output.
"""
import sys, os

sys.path.insert(0, "/opt/trn_rl_repo")

KSTAGE = int(os.environ.get("KSTAGE", "7"))
KSTATS = int(os.environ.get("KSTATS", "1"))

import numpy as np
import ml_dtypes

import concourse.bass as bass
import concourse.tile as tile
from concourse import bacc, mybir
from concourse.bass_utils import run_bass_kernel_spmd

f32 = mybir.dt.float32
bf16 = mybir.dt.bfloat16
i32 = mybir.dt.int32
i16 = mybir.dt.int16
u8 = mybir.dt.uint8
alu = mybir.AluOpType
act = mybir.ActivationFunctionType

B, C, O, H, W = 8, 256, 256, 64, 64
HW = H * W
K = 9
GROUPS = 16
EPS = 1e-5
NT = 8          # pixel tiles per image
TS = 512        # pixels per tile
Q = K * NT      # 72 packed rows
NPERG = (O // GROUPS) * HW  # elements per group = 16*4096


def _emit(nc, tc):
    xt = nc.declare_dram_parameter("xt", [HW + 2, C], bf16, isOutput=False)
    off = nc.declare_dram_parameter("off", [18, HW], f32, isOutput=False)
    msk = nc.declare_dram_parameter("msk", [K, HW], f32, isOutput=False)
    wT = nc.declare_dram_parameter("wT", [18, 128, O], bf16, isOutput=False)
    gbp = nc.declare_dram_parameter("gb", [O, 2], f32, isOutput=False)
    cby = nc.declare_dram_parameter("cby", [Q, TS], f32, isOutput=False)
    cbx = nc.declare_dram_parameter("cbx", [Q, TS], f32, isOutput=False)
    g16 = nc.declare_dram_parameter("g16", [128, 8], f32, isOutput=False)
    g16t = nc.declare_dram_parameter("g16t", [8, 128], f32, isOutput=False)
    idn = nc.declare_dram_parameter("idn", [128, 128], bf16, isOutput=False)
    idnf = nc.declare_dram_parameter("idnf", [128, 128], f32, isOutput=False)
    idni = nc.declare_dram_parameter("idni", [128, 128], u8, isOutput=False)
    outp = nc.declare_dram_parameter("out", [O, HW], f32, isOutput=True)

    dv = nc.vector
    sc = nc.scalar
    pe = nc.tensor
    gs = nc.gpsimd

    with (
        tc.tile_pool(name="const", bufs=1) as constp,
        tc.tile_pool(name="math", bufs=1) as mathp,
        tc.tile_pool(name="gat", bufs=3) as gatp,
        tc.tile_pool(name="val", bufs=2) as valp,
        tc.tile_pool(name="vts", bufs=2) as vtsp,
        tc.tile_pool(name="big", bufs=1) as bigp,
        tc.tile_pool(name="ps", bufs=2, space="PSUM") as psp,
        tc.tile_pool(name="psw", bufs=1, space="PSUM") as pswp,
        tc.tile_pool(name="pst", bufs=2, space="PSUM") as pstp,
    ):
        # ---- constant loads ----
        wTs = constp.tile([128, 18, O], bf16)
        gs.dma_start(wTs[:], wT.ap().transpose([1, 0, 2]))
        idnb = constp.tile([128, 128], bf16)
        gs.dma_start(idnb[:], idn.ap())
        idnft = constp.tile([128, 128], f32)
        gs.dma_start(idnft[:], idnf.ap())
        idnis = constp.tile([128, 128], u8)
        gs.dma_start(idnis[:], idni.ap())
        g16s = constp.tile([128, 8], f32)
        gs.dma_start(g16s[:], g16.ap())
        g16ts = constp.tile([8, 128], f32)
        gs.dma_start(g16ts[:], g16t.ap())
        gbs = constp.tile([128, 2, 2], f32)
        gs.dma_start(gbs[:], gbp.ap().rearrange("(m p) two -> p m two", m=2))

        # ---- packed [72, 512] loads of dy/dx/mask and index ramps ----
        def packed_load(name, src_ap):
            t = mathp.tile([Q, TS], f32, tag=name, name=name)
            gs.dma_start(t[:], src_ap)
            return t

        offv = off.ap().rearrange("(k two) (t s) -> two k t s", two=2, t=NT)
        dys = packed_load("dys", offv[0])
        dxs = packed_load("dxs", offv[1])
        msks = packed_load("msks", msk.ap().rearrange("k (t s) -> k t s", t=NT))
        cbys = packed_load("cbys", cby.ap())
        cbxs = packed_load("cbxs", cbx.ap())

        # Scratch-tag aliasing: transient [72,512] temporaries share slots
        # (Tile inserts WAR deps on reuse; all are sequential DVE ops anyway).
        TAGMAP = {
            "ys": "tA", "yi": "ti", "yf": "tB", "yo": "tC",
            "xs": "tA", "xi": "ti", "xf": "tB", "xo": "tC",
            "yb": "tA", "xlc": "tB", "xrc": "tC",
            "vt": "u1", "vb": "u2", "vl": "u3", "vr": "u4",
            "wyt": "tD", "wxl": "tE",
            "wa": "u5", "wb": "u6",
            "cl": "u1", "cm": "u3", "dr": "u2", "dm": "u4",
            "flat": "tA", "flat2": "tB",
        }

        def mtile(tag, dt=f32):
            tag = TAGMAP.get(tag, tag)
            return mathp.tile([Q, TS], dt, tag=tag, name=tag)

        # ---- floor + frac (robust to cast rounding mode) ----
        def floor_frac(base, d, pre):
            s = mtile(pre + "s")
            dv.tensor_tensor(out=s[:], in0=base[:], in1=d[:], op=alu.add)
            ii = mtile(pre + "i", i32)
            dv.tensor_copy(ii[:], s[:])
            ff = mtile(pre + "f")
            dv.tensor_copy(ff[:], ii[:])
            ov = mtile(pre + "o")
            dv.tensor_tensor(out=ov[:], in0=ff[:], in1=s[:], op=alu.is_gt)
            f0 = mtile(pre + "0")
            dv.tensor_tensor(out=f0[:], in0=ff[:], in1=ov[:], op=alu.subtract)
            fr = mtile(pre + "r")
            dv.tensor_tensor(out=fr[:], in0=s[:], in1=f0[:], op=alu.subtract)
            return f0, fr  # integer part (shifted by +16), fraction in [0,1)

        y0, wy = floor_frac(cbys, dys, "y")
        x0, wx = floor_frac(cbxs, dxs, "x")

        def clamp(src, lo, hi, tag):
            t = mtile(tag)
            dv.tensor_scalar(t[:], src[:], float(lo), float(hi), alu.max, alu.min)
            return t

        y0c = clamp(y0, 16, 79, "y0c")
        yb = mtile("yb")
        dv.tensor_scalar(yb[:], y0[:], 1.0, None, alu.add)
        ybc = clamp(yb, 16, 79, "ybc")
        x0c = clamp(x0, 15, 79, "x0c")   # gather clamp (real -1 allowed: R tap)
        xlc = clamp(x0, 16, 79, "xlc")   # left-tap validity clamp
        xrc = clamp(x0, 15, 78, "xrc")   # right-tap validity clamp

        def is_eq(a, b, tag):
            t = mtile(tag)
            dv.tensor_tensor(out=t[:], in0=a[:], in1=b[:], op=alu.is_equal)
            return t

        vt = is_eq(y0, y0c, "vt")
        vb = is_eq(yb, ybc, "vb")
        vl = is_eq(x0, xlc, "vl")
        vr = is_eq(x0, xrc, "vr")

        wyt = mtile("wyt")
        dv.tensor_scalar(wyt[:], wy[:], -1.0, 1.0, alu.mult, alu.add)
        wxl = mtile("wxl")
        dv.tensor_scalar(wxl[:], wx[:], -1.0, 1.0, alu.mult, alu.add)

        def tmul(a, b, tag):
            t = mtile(tag)
            dv.tensor_tensor(out=t[:], in0=a[:], in1=b[:], op=alu.mult)
            return t

        wa = tmul(wyt, vt, "wa")      # top row weight * validity
        wb = tmul(wy, vb, "wb")       # bottom
        cl = tmul(wxl, vl, "cl")
        cm = tmul(cl, msks, "cm")     # left col weight * validity * mask
        dr = tmul(wx, vr, "dr")
        dm = tmul(dr, msks, "dm")
        w4 = [
            tmul(wa, cm, "wtl"),
            tmul(wa, dm, "wtr"),
            tmul(wb, cm, "wbl"),
            tmul(wb, dm, "wbr"),
        ]

        # ---- flat pair-row indices (with +1 lead-pad row) ----
        # dma_gather wants idx i (= tb*512 + t) at wrapped position
        # [pp = i%16, col = i//16 = tb*32 + t//16], int16, replicated across
        # all eight 16-partition groups.  Write the permuted row on DVE,
        # bounce through DRAM, reload replicated as [128, Q, 64].
        idx16 = bigp.tile([Q, 2 * TS], i16)
        idx16v = idx16[:].rearrange("q (pp s) -> q pp s", pp=16)
        for row, ysrc in ((0, y0c), (1, ybc)):
            ftmp = mtile("flat")
            dv.scalar_tensor_tensor(
                out=ftmp[:], in0=ysrc[:], scalar=64.0, in1=x0c[:],
                op0=alu.mult, op1=alu.add,
            )
            f2 = mtile("flat2")
            dv.tensor_scalar(f2[:], ftmp[:], -1039.0, None, alu.add)
            dv.tensor_copy(idx16v[:, :, row * 32:(row + 1) * 32],
                           f2[:].rearrange("q (a pp) -> q pp a", pp=16))
        idxd = nc.dram_tensor("idxd", [Q, 2 * TS], i16)
        gs.dma_start(idxd.ap(), idx16[:])
        idxw = bigp.tile([128, Q, 64], i16)
        idxdv = idxd.ap().rearrange("q (pp s) -> pp q s", pp=16)
        for grp in range(8):
            gs.dma_start(idxw[grp * 16:(grp + 1) * 16], idxdv)

        # ---- transpose tap weights to [pixel-in-128, (tap, j, q)] ----
        wts = bigp.tile([128, 4, 4, Q], f32)
        for t in range(4):
            for j in range(4):
                pw = pswp.tile([128, Q], f32, tag="pw")
                pe.transpose(pw[:], w4[t][:, j * 128:(j + 1) * 128], idnft[:Q, :Q])
                sc.activation(wts[:, t, j, :], pw[:], act.Copy)
        wtsb = bigp.tile([128, 4, 4, Q], bf16)
        dv.tensor_copy(wtsb[:], wts[:])
        # Persistent ping-pong diag tiles; off-diagonal zeroed once, the
        # per-(k,T) copy_predicated only rewrites diagonal entries.
        dt0 = bigp.tile([128, 128, 16], bf16)
        dt1 = bigp.tile([128, 128, 16], bf16)
        gs.memset(dt0[:], 0.0)
        gs.memset(dt1[:], 0.0)
        dts = (dt0, dt1)

        # ---- stats accumulators ----
        stats = bigp.tile([128, 2, 2, NT], f32)
        out_sb = bigp.tile([128, 2, HW], f32)
        sqscr = mathp.tile([128, TS], f32, tag="sqscr")
        dv.memset(sqscr[:], 0.0)

        if KSTAGE < 6:
            dv.memset(out_sb[:], 0.0)
        # ---- main loop ----
        for T in range(NT):
            vts = vtsp.tile([128, K, 2, TS], bf16, tag="vts")
            if KSTAGE >= 5:
                pass
            for k in range(K if KSTAGE >= 3 else 0):
                q = k * NT + T
                g = gatp.tile([128, 8, TS], bf16, tag="g")
                gs.dma_gather(
                    out_ap=g[:],
                    in_ap=bass.AP(xt.ap().tensor, 0, [[256, 4097], [1, 512]]),
                    idxs_ap=idxw[:, q, :],
                    num_idxs=1024, num_idxs_reg=1024,
                    elem_size=512, elem_step=256,
                )
                if KSTAGE < 4:
                    continue
                dt = dts[(T * K + k) % 2]
                dv.copy_predicated(
                    dt[:].rearrange("p c t -> p t c"),
                    idnis[:].unsqueeze(1).broadcast_to([128, 16, 128]),
                    wtsb[:, :, :, q].rearrange("p a b -> p (a b)")
                        .unsqueeze(2).broadcast_to([128, 16, 128]),
                )
                if KSTAGE < 5:
                    continue
                for ch in range(2):
                    psA = pstp.tile([128, 4, 128], f32, tag="pst")
                    for j in range(4):
                        for t in range(4):
                            lhsT = g[:, (t // 2) * 4 + j,
                                     (t % 2) * 256 + ch * 128:
                                     (t % 2) * 256 + (ch + 1) * 128]
                            pe.matmul(psA[:, j, :], lhsT, dt[:, :, t * 4 + j],
                                      start=(t == 0), stop=(t == 3))
                    sc.activation(vts[:, k, ch, :],
                                  psA[:].rearrange("p a b -> p (a b)"), act.Copy)
            for m in range(2 if KSTAGE >= 6 else 0):
                pso = psp.tile([128, TS], f32, tag="pso")
                for i in range(18):
                    k, ch = i // 2, i % 2
                    pe.matmul(
                        pso[:],
                        wTs[:, i, m * 128:(m + 1) * 128],
                        vts[:, k, ch, :],
                        start=(i == 0),
                        stop=(i == 17),
                    )
                osl = out_sb[:, m, T * TS:(T + 1) * TS]
                sc.activation(osl, pso[:], act.Copy)
                if KSTATS:
                    dv.tensor_reduce(stats[:, m, 0, T:T + 1], osl,
                                     mybir.AxisListType.X, alu.add)
                    dv.tensor_tensor(out=sqscr[:], in0=osl, in1=osl, op=alu.mult)
                    dv.tensor_reduce(stats[:, m, 1, T:T + 1], sqscr[:],
                                     mybir.AxisListType.X, alu.add)

        if KSTAGE < 7:
            for m in range(2):
                for T in range(NT):
                    gs.dma_start(outp.ap()[m * 128:(m + 1) * 128,
                                           T * TS:(T + 1) * TS],
                                 out_sb[:, m, T * TS:(T + 1) * TS])
        # ---- finalize GroupNorm ----
        for m in range(2 if KSTAGE >= 7 else 0):
            tot = mathp.tile([128, 2], f32, tag="tot")
            dv.tensor_reduce(tot[:], stats[:, m, :, :], mybir.AxisListType.X, alu.add)
            psg = pswp.tile([8, 2], f32, tag="psg")
            pe.matmul(psg[:], g16s[:], tot[:], start=True, stop=True)
            sg = mathp.tile([8, 2], f32, tag="sg")
            dv.tensor_copy(sg[:], psg[:])
            mug = mathp.tile([8, 2], f32, tag="mug")  # [:,0]=mu, [:,1]=rs
            dv.tensor_scalar(mug[:, 0:1], sg[:, 0:1], 1.0 / NPERG, None, alu.mult)
            msq = mathp.tile([8, 1], f32, tag="msq")
            dv.tensor_scalar(msq[:], sg[:, 1:2], 1.0 / NPERG, None, alu.mult)
            var = mathp.tile([8, 1], f32, tag="var")
            dv.tensor_tensor(out=var[:], in0=mug[:, 0:1], in1=mug[:, 0:1],
                             op=alu.mult)
            dv.tensor_tensor(out=var[:], in0=msq[:], in1=var[:], op=alu.subtract)
            dv.tensor_scalar(var[:], var[:], EPS, None, alu.add)
            sd = mathp.tile([8, 1], f32, tag="sd", name="sd")
            sc.activation(sd[:], var[:], act.Sqrt)
            dv.reciprocal(mug[:, 1:2], sd[:])
            pse = pswp.tile([128, 2], f32, tag="pse")
            pe.matmul(pse[:], g16ts[:], mug[:], start=True, stop=True)
            ce = mathp.tile([128, 2], f32, tag="ce")
            dv.tensor_copy(ce[:], pse[:])
            scv = mathp.tile([128, 1], f32, tag="scv")
            dv.tensor_tensor(out=scv[:], in0=ce[:, 1:2], in1=gbs[:, m, 0:1],
                             op=alu.mult)
            shv = mathp.tile([128, 1], f32, tag="shv")
            dv.tensor_tensor(out=shv[:], in0=ce[:, 0:1], in1=scv[:], op=alu.mult)
            dv.tensor_tensor(out=shv[:], in0=gbs[:, m, 1:2], in1=shv[:],
                             op=alu.subtract)
            for T in range(NT):
                fin = mathp.tile([128, TS], f32, tag="fin")
                dv.scalar_tensor_tensor(
                    out=fin[:], in0=out_sb[:, m, T * TS:(T + 1) * TS],
                    scalar=scv[:], in1=shv[:].broadcast_to([128, TS]),
                    op0=alu.mult, op1=alu.add,
                )
                gs.dma_start(outp.ap()[m * 128:(m + 1) * 128,
                                       T * TS:(T + 1) * TS], fin[:])


def build_nc():
    nc = bacc.Bacc("TRN2", target_bir_lowering=False, debug=False, num_devices=B)
    with tile.TileContext(nc) as tc:
        _emit(nc, tc)
    nc.compile()
    return nc


def host_inputs(x, offset, mask, weight, gamma, beta):
    """Per-core input maps (layout transforms only)."""
    wt = (
        weight.reshape(O, C, K).transpose(2, 1, 0)
        .reshape(K, 2, 128, O).reshape(18, 128, O)
        .astype(ml_dtypes.bfloat16)
    )
    gb = np.stack([gamma, beta], axis=-1).astype(np.float32)
    n = np.arange(HW)
    h = n // W
    w = n % W
    kk = np.arange(K)
    ky = kk // 3 - 1
    kx = kk % 3 - 1
    cby = (h[None, :] + ky[:, None] + 16.0).astype(np.float32).reshape(Q, TS)
    cbx = (w[None, :] + kx[:, None] + 16.0).astype(np.float32).reshape(Q, TS)
    ii = np.arange(128)
    g16 = (ii[:, None] // 16 == np.arange(8)[None, :]).astype(np.float32)
    g16t = np.ascontiguousarray(g16.T)
    idn = np.eye(128, dtype=ml_dtypes.bfloat16)
    idnf = np.eye(128, dtype=np.float32)
    maps = []
    for b in range(B):
        xtb = np.zeros((HW + 2, C), dtype=ml_dtypes.bfloat16)
        xtb[1:HW + 1] = x[b].reshape(C, HW).T.astype(ml_dtypes.bfloat16)
        maps.append({
            "xt": xtb,
            "off": np.ascontiguousarray(offset[b].reshape(18, HW), np.float32),
            "msk": np.ascontiguousarray(mask[b].reshape(K, HW), np.float32),
            "wT": wt, "gb": gb, "cby": cby, "cbx": cbx,
            "g16": g16, "g16t": g16t, "idn": idn, "idnf": idnf,
            "idni": np.eye(128, dtype=np.uint8),
        })
    return maps


def finalize_output(stacked):
    """Per-core 'out' [B, O, HW] -> full [B, O, H, W] float32."""
    return np.asarray(stacked).reshape(B, O, H, W).astype(np.float32)


_NC_CACHE = None


def kernel(x, offset, mask, weight, gamma, beta):
    global _NC_CACHE
    x = np.asarray(x)
    offset = np.asarray(offset)
    mask = np.asarray(mask)
    weight = np.asarray(weight)
    gamma = np.asarray(gamma)
    beta = np.asarray(beta)
    if _NC_CACHE is None:
        _NC_CACHE = build_nc()
    nc = _NC_CACHE
    maps = host_inputs(x, offset, mask, weight, gamma, beta)
    res = run_bass_kernel_spmd(nc, maps, list(range(B)))
    out = np.stack([res.results[b]["out"] for b in range(B)], axis=0)
    return finalize_output(out)


if __name__ == "__main__":
    pass

